# revision 20
# baseline (speedup 1.0000x reference)
"""Trainium2 Bass kernel for a single-step seq2seq GRU decoder with attention.

Computation (batch=1):
  embedded = emb[prev_word]                                      [1, H]
  attn_w   = softmax([embedded, h_prev] @ attn_W.T + attn_b)     [1, L]
  focused  = attn_w @ encoder_outputs                            [1, H]
  gru_in   = relu([focused, embedded] @ comb_W.T + comb_b)       [1, H]
  h_new    = GRU(gru_in, h_prev)                                 [1, H]
  probs    = log_softmax(h_new @ out_W.T + out_b)                [1, V]

Distribution over 8 NeuronCores, with NO cross-core collectives:
  - the vocab dim of out_W/out_b (the 206MB input that dominates the
    memory-bound roofline) is sharded 8 ways
  - the small attention/comb/GRU stages are replicated on every core
    (batch=1: cheaper than paying a cross-core sync for their shards)
  - log-softmax: each core emits its local exp-sum; the host unshard step
    combines the 8 scalars and subtracts log(S) while concatenating.
  Collectives are deliberately avoided: a NEFF with collectives pays a
  multi-core rendezvous at entry, which costs the full inter-core dispatch
  skew on every execution.

On-device layout: a hidden vector x[1024] lives as [128, 8] SBUF tiles with
x[c*128+p] at [p, c] (partition-parallel everywhere; no transposes needed).
"""

import os
import numpy as np
import ml_dtypes

import concourse.bass as bass
import concourse.bacc as bacc
import concourse.mybir as mybir
import concourse.tile as tile
import concourse.bass_utils as bass_utils

V, H, L = 50257, 1024, 20
N_CORES = 8
P = 128
KC = H // P            # 8 hidden chunks of 128
VP = 51200             # padded vocab = 8 * 6400
VS = VP // N_CORES     # 6400 vocab rows per core
VC = VS // P           # 50 vocab chunks of 128

F32 = mybir.dt.float32
BF16 = mybir.dt.bfloat16
NEG_BIG = -1.0e30

# dtype of the replicated comb/GRU weights (WDT) and the out_W shard (ODT).
# GRU stays bf16 (fp8 would push h_new error to ~1.5e-2); out_W tolerates
# fp8 e4m3 with a 256x scale (probs error ~3e-3 vs bf16's 3e-4).
WDT = BF16
WDT_NP = ml_dtypes.bfloat16
OUT_FP8 = True
ODT = mybir.dt.float8e4 if OUT_FP8 else BF16
ODT_NP = ml_dtypes.float8_e4m3 if OUT_FP8 else ml_dtypes.bfloat16
W_SCALE = 256.0 if OUT_FP8 else 1.0   # fp8 quantization scale for out_W
H_SCALE = 16.0 if OUT_FP8 else 1.0    # fp8 quantization scale for h_new rhs
UNSCALE = 1.0 / (W_SCALE * H_SCALE)


def build_nc():
    nc = bacc.Bacc(
        "TRN2",
        target_bir_lowering=False,
        debug=False,
        enable_asserts=False,
        num_devices=N_CORES,
    )

    def inp(name, shape, dt=F32):
        return nc.dram_tensor(name, shape, dt, kind="ExternalInput").ap()

    # replicated inputs
    emb_pc = inp("emb_pc", [P, KC])            # embedded word, (c p) -> p c
    hprev_pc = inp("hprev_pc", [P, KC])        # h_prev, (c p) -> p c
    attn_wt = inp("attn_wt", [P, 16 * L])      # attn_W.T chunked [p, c*L+m]
    attn_b = inp("attn_b", [L, 1])
    enc = inp("enc", [L, H])                   # encoder_outputs
    comb_wt = inp("comb_wt", [P, 16 * KC * P], WDT)  # [p,(co*16+ck)*128+m]
    comb_b = inp("comb_b", [P, KC])
    wih_t = inp("wih_t", [P, 3 * KC * H], WDT)  # [p,((g*8+co)*8+kc)*128+m]
    whh_t = inp("whh_t", [P, 3 * KC * H], WDT)
    brz = inp("brz", [P, 2 * KC])              # (b_ih+b_hh) r,z in (c p)
    b_in = inp("b_in", [P, KC])                # b_ih n slice
    b_hn = inp("b_hn", [P, KC])                # b_hh n slice
    # sharded inputs. out_wt is packed for fp8 DoubleRow: pair-chunk kc4
    # holds hidden rows [256*kc4, 256*kc4+256) as [p, i, v] = row 128i+p.
    out_wt = inp("out_wt", [KC // 2, P, 2, VS], ODT)
    out_b = inp("out_b", [P, VC])              # bias shard [p, vc]

    logits_out = nc.dram_tensor("logits", [P, VC], F32, kind="ExternalOutput").ap()
    ssum_out = nc.dram_tensor("ssum", [1, 1], F32, kind="ExternalOutput").ap()
    hnew_out = nc.dram_tensor("h_new", [P, KC], F32, kind="ExternalOutput").ap()

    with tile.TileContext(nc) as tc:
        with (
            tc.tile_pool(name="consts", bufs=1) as consts,
            tc.tile_pool(name="sb", bufs=1) as sb,
            tc.tile_pool(name="wpool", bufs=1) as wpool,
            tc.tile_pool(name="psum", bufs=1, space="PSUM") as psum,
        ):
            ones = consts.tile([P, P], F32)
            nc.vector.memset(ones[:], 1.0)

            # ---- small/critical weights first (DMA order matters) --------
            emb_sb = sb.tile([P, KC], F32)
            nc.sync.dma_start(emb_sb[:], emb_pc)
            hprev_sb = sb.tile([P, KC], F32)
            nc.sync.dma_start(hprev_sb[:], hprev_pc)
            attn_wt_sb = sb.tile([P, 16 * L], F32)
            nc.sync.dma_start(attn_wt_sb[:], attn_wt)
            attn_b_sb = sb.tile([L, 1], F32)
            nc.sync.dma_start(attn_b_sb[:], attn_b)
            enc_sb = sb.tile([L, H], F32)
            nc.sync.dma_start(enc_sb[:], enc)
            comb_b_sb = sb.tile([P, KC], F32)
            nc.sync.dma_start(comb_b_sb[:], comb_b)
            brz_sb = sb.tile([P, 2 * KC], F32)
            nc.sync.dma_start(brz_sb[:], brz)
            b_in_sb = sb.tile([P, KC], F32)
            nc.sync.dma_start(b_in_sb[:], b_in)
            b_hn_sb = sb.tile([P, KC], F32)
            nc.sync.dma_start(b_hn_sb[:], b_hn)
            out_b_sb = sb.tile([P, VC], F32)
            nc.sync.dma_start(out_b_sb[:], out_b)

            # per-gate GRU weight tiles so gate matmuls start as each lands;
            # whh first (gh depends only on h_prev, runs before the
            # attention->comb chain finishes)
            GSZ = KC * KC * P  # one gate's blocks: 8 out-chunks x 8 k-chunks
            whh_g = []
            for g in range(3):
                t = sb.tile([P, GSZ], WDT, tag=f"whh{g}")
                nc.sync.dma_start(t[:], whh_t[:, g * GSZ : (g + 1) * GSZ])
                whh_g.append(t)
            comb_wt_sb = sb.tile([P, 16 * KC * P], WDT)
            nc.sync.dma_start(comb_wt_sb[:], comb_wt)
            wih_g = []
            for g in range(3):
                t = sb.tile([P, GSZ], WDT, tag=f"wih{g}")
                nc.sync.dma_start(t[:], wih_t[:, g * GSZ : (g + 1) * GSZ])
                wih_g.append(t)

            # ---- big out_W shard: 4 resident DoubleRow pair-chunk tiles --
            w_sb = []
            for kc4 in range(KC // 2):
                wt = wpool.tile([P, 2, VS], ODT, tag=f"w{kc4}")
                nc.sync.dma_start(wt[:, 0, :], out_wt[kc4][:, 0, :])
                nc.sync.dma_start(wt[:, 1, :], out_wt[kc4][:, 1, :])
                w_sb.append(wt)

            # ---- stage A: attention (replicated, fp32) -------------------
            attnlog_ps = psum.tile([L, 1], F32, tag="pa")
            for c in range(16):
                rhs = emb_sb[:, c : c + 1] if c < KC else hprev_sb[:, c - KC : c - KC + 1]
                nc.tensor.matmul(
                    attnlog_ps[:],
                    attn_wt_sb[:, c * L : (c + 1) * L],
                    rhs,
                    start=(c == 0),
                    stop=(c == 15),
                )
            # exp(logit + b); logits are tiny so no max-subtraction needed
            expw_sb = sb.tile([L, 1], F32)
            nc.scalar.activation(
                expw_sb[:], attnlog_ps[:], mybir.ActivationFunctionType.Exp,
                bias=attn_b_sb[:],
            )
            asum_ps = psum.tile([1, 1], F32, tag="pb")
            nc.tensor.matmul(asum_ps[:], expw_sb[:], ones[:L, 0:1], start=True, stop=True)
            arecip_sb = sb.tile([1, 1], F32)
            nc.vector.reciprocal(arecip_sb[:], asum_ps[:])
            # focused (unnormalized) [128, KC]
            foc_ps = psum.tile([P, KC], F32, tag="pm")
            for c in range(KC):
                nc.tensor.matmul(
                    foc_ps[:, c : c + 1],
                    enc_sb[:, c * P : (c + 1) * P],
                    expw_sb[:],
                    start=True,
                    stop=True,
                )
            # broadcast 1/denom across partitions via PE ones column
            arb_ps = psum.tile([P, 1], F32, tag="pb")
            nc.tensor.matmul(arb_ps[:], ones[0:1, :], arecip_sb[:], start=True, stop=True)
            arb_sb = sb.tile([P, 1], F32)
            nc.scalar.copy(arb_sb[:], arb_ps[:])
            fsc_sb = sb.tile([P, KC], WDT)
            nc.vector.tensor_scalar_mul(fsc_sb[:], foc_ps[:], arb_sb[:])
            emb_w_sb = sb.tile([P, KC], WDT)
            nc.vector.tensor_copy(emb_w_sb[:], emb_sb[:])

            # ---- stage B: comb (replicated) -> gru_in [128, KC] ----------
            gcol_ps = psum.tile([P, KC], F32, tag="pm")
            for co in range(KC):
                for ck in range(16):
                    rhs = (fsc_sb[:, ck : ck + 1] if ck < KC
                           else emb_w_sb[:, ck - KC : ck - KC + 1])
                    nc.tensor.matmul(
                        gcol_ps[:, co : co + 1],
                        comb_wt_sb[:, (co * 16 + ck) * P : (co * 16 + ck + 1) * P],
                        rhs,
                        start=(ck == 0),
                        stop=(ck == 15),
                    )
            gin_f_sb = sb.tile([P, KC], F32)
            nc.vector.tensor_add(gin_f_sb[:], gcol_ps[:], comb_b_sb[:])
            nc.vector.tensor_relu(gin_f_sb[:], gin_f_sb[:])
            gin_sb = sb.tile([P, KC], WDT)
            nc.vector.tensor_copy(gin_sb[:], gin_f_sb[:])
            hprev_w_sb = sb.tile([P, KC], WDT)
            nc.vector.tensor_copy(hprev_w_sb[:], hprev_sb[:])

            # ---- stages C+D pipelined per h_new column pair --------------
            # For each pair of h_new columns (2*co4, 2*co4+1): run the GRU
            # matmuls and gate math for just those columns, then immediately
            # the 50 DoubleRow vocab matmuls that consume them. The vocab
            # projection overlaps the remaining GRU work instead of waiting
            # for the full h_new.
            hnew_sb = sb.tile([P, KC], F32)
            hnew_w_sb = sb.tile([P, KC], ODT)
            logits_sb = sb.tile([P, VC], F32)
            for co4 in range(KC // 2):
                cs = slice(2 * co4, 2 * co4 + 2)
                # gi/gh for the pair: psum [P, 6], col j = g*2 + cc
                gh_ps = psum.tile([P, 6], F32, tag="pgh")
                gi_ps = psum.tile([P, 6], F32, tag="pgi")
                for g in range(3):
                    for cc in range(2):
                        co = 2 * co4 + cc
                        for kc in range(KC):
                            off = (co * KC + kc) * P
                            nc.tensor.matmul(
                                gh_ps[:, g * 2 + cc : g * 2 + cc + 1],
                                whh_g[g][:, off : off + P],
                                hprev_w_sb[:, kc : kc + 1],
                                start=(kc == 0),
                                stop=(kc == KC - 1),
                            )
                for g in range(3):
                    for cc in range(2):
                        co = 2 * co4 + cc
                        for kc in range(KC):
                            off = (co * KC + kc) * P
                            nc.tensor.matmul(
                                gi_ps[:, g * 2 + cc : g * 2 + cc + 1],
                                wih_g[g][:, off : off + P],
                                gin_sb[:, kc : kc + 1],
                                start=(kc == 0),
                                stop=(kc == KC - 1),
                            )
                gh_sb = sb.tile([P, 6], F32, tag="ghc")
                nc.scalar.copy(gh_sb[:], gh_ps[:])
                # r,z = sigmoid(gi + gh + brz) (cols 0:4 = r0 r1 z0 z1)
                rz_sb = sb.tile([P, 4], F32, tag="rzc")
                nc.vector.tensor_add(rz_sb[:], gi_ps[:, 0:4], gh_sb[:, 0:4])
                nc.vector.tensor_add(rz_sb[:], rz_sb[:], brz_sb[:, 4 * co4 : 4 * co4 + 4])
                nc.scalar.activation(
                    rz_sb[:], rz_sb[:], mybir.ActivationFunctionType.Sigmoid)
                # n = tanh(gi_n + b_in + r*(gh_n + b_hn))
                hnb_sb = sb.tile([P, 2], F32, tag="hnbc")
                nc.vector.tensor_add(hnb_sb[:], gh_sb[:, 4:6], b_hn_sb[:, cs])
                nc.vector.tensor_mul(hnb_sb[:], hnb_sb[:], rz_sb[:, 0:2])
                npre_sb = sb.tile([P, 2], F32, tag="nprec")
                nc.vector.tensor_add(npre_sb[:], gi_ps[:, 4:6], hnb_sb[:])
                nc.vector.tensor_add(npre_sb[:], npre_sb[:], b_in_sb[:, cs])
                n_sb = sb.tile([P, 2], F32, tag="nc")
                nc.scalar.activation(
                    n_sb[:], npre_sb[:], mybir.ActivationFunctionType.Tanh)
                # h_new = n + z*(h - n)
                hmn_sb = sb.tile([P, 2], F32, tag="hmnc")
                nc.vector.tensor_sub(hmn_sb[:], hprev_sb[:, cs], n_sb[:])
                nc.vector.tensor_mul(hmn_sb[:], hmn_sb[:], rz_sb[:, 2:4])
                nc.vector.tensor_add(hnew_sb[:, cs], n_sb[:], hmn_sb[:])
                if OUT_FP8:
                    nc.vector.tensor_scalar_mul(
                        hnew_w_sb[:, cs], hnew_sb[:, cs], H_SCALE)
                else:
                    nc.vector.tensor_copy(hnew_w_sb[:, cs], hnew_sb[:, cs])

                # vocab projection for this pair
                part_ps = psum.tile([P, VC], F32, tag="plog", bufs=2)
                for vc in range(VC):
                    nc.tensor.matmul(
                        part_ps[:, vc : vc + 1],
                        w_sb[co4][:, :, vc * P : (vc + 1) * P],
                        hnew_w_sb[:, cs].rearrange("p (two n) -> p two n", n=1),
                        start=True,
                        stop=True,
                        perf_mode=mybir.MatmulPerfMode.DoubleRow,
                    )
                if co4 == 0:
                    nc.vector.scalar_tensor_tensor(
                        logits_sb[:], part_ps[:], UNSCALE, out_b_sb[:],
                        op0=mybir.AluOpType.mult, op1=mybir.AluOpType.add,
                    )
                else:
                    nc.vector.scalar_tensor_tensor(
                        logits_sb[:], part_ps[:], UNSCALE, logits_sb[:],
                        op0=mybir.AluOpType.mult, op1=mybir.AluOpType.add,
                    )
            nc.sync.dma_start(hnew_out, hnew_sb[:])
            nc.sync.dma_start(logits_out, logits_sb[:])

            # ---- local exp-sum for the host-side log-softmax -------------
            exp_sb = sb.tile([P, VC], F32)
            srow_sb = sb.tile([P, 1], F32)
            nc.scalar.activation(
                exp_sb[:], logits_sb[:], mybir.ActivationFunctionType.Exp,
                accum_out=srow_sb[:],
            )
            ssum_ps = psum.tile([1, 1], F32, tag="pb")
            nc.tensor.matmul(ssum_ps[:], srow_sb[:], ones[:, 0:1], start=True, stop=True)
            ssum_sb = sb.tile([1, 1], F32)
            nc.scalar.copy(ssum_sb[:], ssum_ps[:])
            nc.sync.dma_start(ssum_out, ssum_sb[:])

    nc.compile()
    return nc


_NC_CACHE = None


def _get_nc():
    global _NC_CACHE
    if _NC_CACHE is None:
        _NC_CACHE = build_nc()
    return _NC_CACHE


def _pc(v):
    """[1024] -> [128, 8] with v[c*128+p] at [p, c]."""
    return np.ascontiguousarray(v.reshape(KC, P).T)


def make_in_maps(prev_word, prev_hidden, encoder_outputs, emb, attn_W, attn_b,
                 comb_W, comb_b, w_ih, w_hh, b_ih, b_hh, out_W, out_b):
    f32 = lambda a: np.asarray(a, dtype=np.float32)
    idx = int(np.asarray(prev_word).reshape(-1)[0])
    emb_row = f32(emb)[idx].reshape(H)
    hprev = f32(prev_hidden).reshape(H)
    attn_W = f32(attn_W)
    attn_b = f32(attn_b)
    enc = np.ascontiguousarray(f32(encoder_outputs))
    comb_W = f32(comb_W)
    comb_b = f32(comb_b)
    w_ih, w_hh, b_ih, b_hh = f32(w_ih), f32(w_hh), f32(b_ih), f32(b_hh)
    out_W, out_b = f32(out_W), f32(out_b)

    # replicated tensors (same arrays for every core)
    rep = {
        "emb_pc": _pc(emb_row),
        "hprev_pc": _pc(hprev),
        # [p, c*L+m] = attn_W[m, c*128+p]
        "attn_wt": np.ascontiguousarray(
            attn_W.T.reshape(16, P, L).transpose(1, 0, 2).reshape(P, 16 * L)),
        "attn_b": np.ascontiguousarray(attn_b.reshape(L, 1)),
        "enc": enc,
        # [p, (co*16+ck)*128+m] = comb_W[co*128+m, ck*128+p]
        "comb_wt": np.ascontiguousarray(
            comb_W.reshape(KC, P, 16, P).transpose(3, 0, 2, 1)
            .reshape(P, 16 * KC * P)).astype(WDT_NP),
        "comb_b": _pc(comb_b),
        # [p, ((g*8+co)*8+kc)*128+m] = W[g*1024+co*128+m, kc*128+p]
        "wih_t": np.ascontiguousarray(
            w_ih.reshape(3, KC, P, KC, P).transpose(4, 0, 1, 3, 2)
            .reshape(P, 3 * KC * H)).astype(WDT_NP),
        "whh_t": np.ascontiguousarray(
            w_hh.reshape(3, KC, P, KC, P).transpose(4, 0, 1, 3, 2)
            .reshape(P, 3 * KC * H)).astype(WDT_NP),
        # pair-major r/z bias: block co4 = [r_{2co4}, r_{2co4+1}, z_{2co4}, z_{2co4+1}]
        "brz": np.ascontiguousarray(np.concatenate(
            [_pc((b_ih + b_hh)[:H]).reshape(P, KC // 2, 2),
             _pc((b_ih + b_hh)[H : 2 * H]).reshape(P, KC // 2, 2)],
            axis=2).reshape(P, 2 * KC)),
        "b_in": _pc(b_ih[2 * H :]),
        "b_hn": _pc(b_hh[2 * H :]),
    }

    Wp = np.zeros((VP, H), np.float32)
    Wp[:V] = out_W
    bp = np.full(VP, NEG_BIG, np.float32)
    bp[:V] = out_b

    in_maps = []
    for i in range(N_CORES):
        vsl = slice(VS * i, VS * (i + 1))
        m = dict(rep)
        m["out_wt"] = np.ascontiguousarray(
            np.clip(Wp[vsl].T * W_SCALE, -240.0, 240.0)
            .reshape(KC // 2, 2, P, VS).transpose(0, 2, 1, 3)
        ).astype(ODT_NP)
        m["out_b"] = np.ascontiguousarray(bp[vsl].reshape(VC, P).T)
        in_maps.append(m)
    return in_maps


LAST_RESULTS = None


def kernel(**inputs):
    global LAST_RESULTS
    nc = _get_nc()
    in_maps = make_in_maps(**inputs)
    trace = bool(int(os.environ.get("KERNEL_TRACE", "0")))
    res = bass_utils.run_bass_kernel_spmd(
        nc, in_maps, core_ids=list(range(N_CORES)), trace=trace,
    )
    LAST_RESULTS = res
    logits = np.concatenate(
        [np.asarray(r["logits"]).T.reshape(VS) for r in res.results])
    s_total = float(sum(np.asarray(r["ssum"]).reshape(()) for r in res.results))
    probs = (logits[:V] - np.float32(np.log(s_total))).reshape(1, V)
    h_new = np.asarray(res.results[0]["h_new"]).T.reshape(1, 1, H)
    return probs.astype(np.float32), h_new.astype(np.float32)


# revision 23
# speedup vs baseline: 1.0797x; 1.0797x over previous
"""Trainium2 Bass kernel for a single-step seq2seq GRU decoder with attention.

Computation (batch=1):
  embedded = emb[prev_word]                                      [1, H]
  attn_w   = softmax([embedded, h_prev] @ attn_W.T + attn_b)     [1, L]
  focused  = attn_w @ encoder_outputs                            [1, H]
  gru_in   = relu([focused, embedded] @ comb_W.T + comb_b)       [1, H]
  h_new    = GRU(gru_in, h_prev)                                 [1, H]
  probs    = log_softmax(h_new @ out_W.T + out_b)                [1, V]

Distribution over 8 NeuronCores, with NO cross-core collectives:
  - the vocab dim of out_W/out_b (the 206MB input that dominates the
    memory-bound roofline) is sharded 8 ways
  - the small attention/comb/GRU stages are replicated on every core
    (batch=1: cheaper than paying a cross-core sync for their shards)
  - log-softmax: each core emits its local exp-sum; the host unshard step
    combines the 8 scalars and subtracts log(S) while concatenating.
  Collectives are deliberately avoided: a NEFF with collectives pays a
  multi-core rendezvous at entry, which costs the full inter-core dispatch
  skew on every execution.

On-device layout: a hidden vector x[1024] lives as [128, 8] SBUF tiles with
x[c*128+p] at [p, c] (partition-parallel everywhere; no transposes needed).
"""

import os
import numpy as np
import ml_dtypes

import concourse.bass as bass
import concourse.bacc as bacc
import concourse.mybir as mybir
import concourse.tile as tile
import concourse.bass_utils as bass_utils

V, H, L = 50257, 1024, 20
N_CORES = 8
P = 128
KC = H // P            # 8 hidden chunks of 128
VP = 51200             # padded vocab = 8 * 6400
VS = VP // N_CORES     # 6400 vocab rows per core
VC = VS // P           # 50 vocab chunks of 128

F32 = mybir.dt.float32
BF16 = mybir.dt.bfloat16
NEG_BIG = -1.0e30

# dtype of the replicated comb/GRU weights (WDT) and the out_W shard (ODT).
# GRU stays bf16 (fp8 would push h_new error to ~1.5e-2); out_W tolerates
# fp8 e4m3 with a 256x scale (probs error ~3e-3 vs bf16's 3e-4).
WDT = BF16
WDT_NP = ml_dtypes.bfloat16
OUT_FP8 = True
ODT = mybir.dt.float8e4 if OUT_FP8 else BF16
ODT_NP = ml_dtypes.float8_e4m3 if OUT_FP8 else ml_dtypes.bfloat16
W_SCALE = 256.0 if OUT_FP8 else 1.0   # fp8 quantization scale for out_W
H_SCALE = 16.0 if OUT_FP8 else 1.0    # fp8 quantization scale for h_new rhs
UNSCALE = 1.0 / (W_SCALE * H_SCALE)


def build_nc():
    nc = bacc.Bacc(
        "TRN2",
        target_bir_lowering=False,
        debug=False,
        enable_asserts=False,
        num_devices=N_CORES,
    )

    def inp(name, shape, dt=F32):
        return nc.dram_tensor(name, shape, dt, kind="ExternalInput").ap()

    # replicated inputs
    emb_pc = inp("emb_pc", [P, KC])            # embedded word, (c p) -> p c
    hprev_pc = inp("hprev_pc", [P, KC])        # h_prev, (c p) -> p c
    attn_wt = inp("attn_wt", [P, 16 * L])      # attn_W.T chunked [p, c*L+m]
    attn_b = inp("attn_b", [L, 1])
    enc = inp("enc", [L, H])                   # encoder_outputs
    comb_wt = inp("comb_wt", [P, 16 * KC * P], WDT)  # [p,(co*16+ck)*128+m]
    comb_b = inp("comb_b", [P, KC])
    wih_t = inp("wih_t", [P, 3 * KC * H], WDT)  # [p,((g*8+co)*8+kc)*128+m]
    whh_t = inp("whh_t", [P, 3 * KC * H], WDT)
    brz = inp("brz", [P, 2 * KC])              # (b_ih+b_hh) r,z in (c p)
    b_in = inp("b_in", [P, KC])                # b_ih n slice
    b_hn = inp("b_hn", [P, KC])                # b_hh n slice
    # sharded inputs. out_wt is packed for fp8 DoubleRow: pair-chunk kc4
    # holds hidden rows [256*kc4, 256*kc4+256) as [p, i, v] = row 128i+p.
    out_wt = inp("out_wt", [KC // 2, P, 2, VS], ODT)
    out_b = inp("out_b", [P, VC])              # bias shard [p, vc]

    logits_out = nc.dram_tensor("logits", [P, VC], F32, kind="ExternalOutput").ap()
    ssum_out = nc.dram_tensor("ssum", [1, 1], F32, kind="ExternalOutput").ap()
    hnew_out = nc.dram_tensor("h_new", [P, KC], F32, kind="ExternalOutput").ap()

    with tile.TileContext(nc) as tc:
        with (
            tc.tile_pool(name="consts", bufs=1) as consts,
            tc.tile_pool(name="sb", bufs=1) as sb,
            tc.tile_pool(name="wpool", bufs=1) as wpool,
            tc.tile_pool(name="psum", bufs=1, space="PSUM") as psum,
        ):
            ones = consts.tile([P, P], F32)
            nc.vector.memset(ones[:], 1.0)

            # ---- small/critical weights first (DMA order matters) --------
            emb_sb = sb.tile([P, KC], F32)
            nc.sync.dma_start(emb_sb[:], emb_pc)
            hprev_sb = sb.tile([P, KC], F32)
            nc.sync.dma_start(hprev_sb[:], hprev_pc)
            attn_wt_sb = sb.tile([P, 16 * L], F32)
            nc.sync.dma_start(attn_wt_sb[:], attn_wt)
            attn_b_sb = sb.tile([L, 1], F32)
            nc.sync.dma_start(attn_b_sb[:], attn_b)
            enc_sb = sb.tile([L, H], F32)
            nc.sync.dma_start(enc_sb[:], enc)
            comb_b_sb = sb.tile([P, KC], F32)
            nc.sync.dma_start(comb_b_sb[:], comb_b)
            brz_sb = sb.tile([P, 2 * KC], F32)
            nc.sync.dma_start(brz_sb[:], brz)
            b_in_sb = sb.tile([P, KC], F32)
            nc.sync.dma_start(b_in_sb[:], b_in)
            b_hn_sb = sb.tile([P, KC], F32)
            nc.sync.dma_start(b_hn_sb[:], b_hn)
            out_b_sb = sb.tile([P, VC], F32)
            nc.sync.dma_start(out_b_sb[:], out_b)

            # GRU weights in pair-major tiles (all 3 gates for one h_new
            # column pair per tile) and out_W pair-chunks, with DMAs
            # interleaved in pipeline consumption order: the pair-k GRU
            # matmuls and vocab matmuls become runnable as early as possible.
            PSZ = 3 * 2 * KC * P  # blocks for one column pair: 3 gates x 2 cols x 8 k
            comb_wt_sb = sb.tile([P, 16 * KC * P], WDT)
            whh_p, wih_p, w_sb = [], [], []
            for co4 in range(KC // 2):
                t = sb.tile([P, PSZ], WDT, tag=f"whh{co4}")
                nc.sync.dma_start(t[:], whh_t[:, co4 * PSZ : (co4 + 1) * PSZ])
                whh_p.append(t)
                if co4 == 0:
                    ch = 16 * KC * P // 2
                    nc.sync.dma_start(comb_wt_sb[:, :ch], comb_wt[:, :ch])
                    nc.sync.dma_start(comb_wt_sb[:, ch:], comb_wt[:, ch:])
                t = sb.tile([P, PSZ], WDT, tag=f"wih{co4}")
                nc.sync.dma_start(t[:], wih_t[:, co4 * PSZ : (co4 + 1) * PSZ])
                wih_p.append(t)
                wt = wpool.tile([P, 2, VS], ODT, tag=f"w{co4}")
                nc.sync.dma_start(wt[:, 0, :], out_wt[co4][:, 0, :])
                nc.sync.dma_start(wt[:, 1, :], out_wt[co4][:, 1, :])
                w_sb.append(wt)

            # ---- stage A: attention (replicated, fp32) -------------------
            attnlog_ps = psum.tile([L, 1], F32, tag="pa")
            for c in range(16):
                rhs = emb_sb[:, c : c + 1] if c < KC else hprev_sb[:, c - KC : c - KC + 1]
                nc.tensor.matmul(
                    attnlog_ps[:],
                    attn_wt_sb[:, c * L : (c + 1) * L],
                    rhs,
                    start=(c == 0),
                    stop=(c == 15),
                )
            # exp(logit + b); logits are tiny so no max-subtraction needed
            expw_sb = sb.tile([L, 1], F32)
            nc.scalar.activation(
                expw_sb[:], attnlog_ps[:], mybir.ActivationFunctionType.Exp,
                bias=attn_b_sb[:],
            )
            asum_ps = psum.tile([1, 1], F32, tag="pb")
            nc.tensor.matmul(asum_ps[:], expw_sb[:], ones[:L, 0:1], start=True, stop=True)
            arecip_sb = sb.tile([1, 1], F32)
            nc.vector.reciprocal(arecip_sb[:], asum_ps[:])
            # focused (unnormalized) [128, KC]
            foc_ps = psum.tile([P, KC], F32, tag="pm")
            for c in range(KC):
                nc.tensor.matmul(
                    foc_ps[:, c : c + 1],
                    enc_sb[:, c * P : (c + 1) * P],
                    expw_sb[:],
                    start=True,
                    stop=True,
                )
            # broadcast 1/denom across partitions via PE ones column
            arb_ps = psum.tile([P, 1], F32, tag="pb")
            nc.tensor.matmul(arb_ps[:], ones[0:1, :], arecip_sb[:], start=True, stop=True)
            arb_sb = sb.tile([P, 1], F32)
            nc.scalar.copy(arb_sb[:], arb_ps[:])
            fsc_sb = sb.tile([P, KC], WDT)
            nc.vector.tensor_scalar_mul(fsc_sb[:], foc_ps[:], arb_sb[:])
            emb_w_sb = sb.tile([P, KC], WDT)
            nc.vector.tensor_copy(emb_w_sb[:], emb_sb[:])

            # ---- stage B: comb (replicated) -> gru_in [128, KC] ----------
            gcol_ps = psum.tile([P, KC], F32, tag="pm")
            for co in range(KC):
                for ck in range(16):
                    rhs = (fsc_sb[:, ck : ck + 1] if ck < KC
                           else emb_w_sb[:, ck - KC : ck - KC + 1])
                    nc.tensor.matmul(
                        gcol_ps[:, co : co + 1],
                        comb_wt_sb[:, (co * 16 + ck) * P : (co * 16 + ck + 1) * P],
                        rhs,
                        start=(ck == 0),
                        stop=(ck == 15),
                    )
            gin_f_sb = sb.tile([P, KC], F32)
            nc.vector.tensor_add(gin_f_sb[:], gcol_ps[:], comb_b_sb[:])
            nc.vector.tensor_relu(gin_f_sb[:], gin_f_sb[:])
            gin_sb = sb.tile([P, KC], WDT)
            nc.vector.tensor_copy(gin_sb[:], gin_f_sb[:])
            hprev_w_sb = sb.tile([P, KC], WDT)
            nc.vector.tensor_copy(hprev_w_sb[:], hprev_sb[:])

            # ---- stages C+D pipelined per h_new column pair --------------
            # For each pair of h_new columns (2*co4, 2*co4+1): run the GRU
            # matmuls and gate math for just those columns, then immediately
            # the 50 DoubleRow vocab matmuls that consume them. The vocab
            # projection overlaps the remaining GRU work instead of waiting
            # for the full h_new.
            hnew_sb = sb.tile([P, KC], F32)
            hnew_w_sb = sb.tile([P, KC], ODT)
            logits_sb = sb.tile([P, VC], F32)
            for co4 in range(KC // 2):
                cs = slice(2 * co4, 2 * co4 + 2)
                # gi/gh for the pair: psum [P, 6], col j = g*2 + cc
                gh_ps = psum.tile([P, 6], F32, tag="pgh")
                gi_ps = psum.tile([P, 6], F32, tag="pgi")
                for g in range(3):
                    for cc in range(2):
                        for kc in range(KC):
                            off = ((g * 2 + cc) * KC + kc) * P
                            nc.tensor.matmul(
                                gh_ps[:, g * 2 + cc : g * 2 + cc + 1],
                                whh_p[co4][:, off : off + P],
                                hprev_w_sb[:, kc : kc + 1],
                                start=(kc == 0),
                                stop=(kc == KC - 1),
                            )
                for g in range(3):
                    for cc in range(2):
                        for kc in range(KC):
                            off = ((g * 2 + cc) * KC + kc) * P
                            nc.tensor.matmul(
                                gi_ps[:, g * 2 + cc : g * 2 + cc + 1],
                                wih_p[co4][:, off : off + P],
                                gin_sb[:, kc : kc + 1],
                                start=(kc == 0),
                                stop=(kc == KC - 1),
                            )
                gh_sb = sb.tile([P, 6], F32, tag="ghc")
                nc.scalar.copy(gh_sb[:], gh_ps[:])
                # r,z = sigmoid(gi + gh + brz) (cols 0:4 = r0 r1 z0 z1)
                rz_sb = sb.tile([P, 4], F32, tag="rzc")
                nc.vector.tensor_add(rz_sb[:], gi_ps[:, 0:4], gh_sb[:, 0:4])
                nc.vector.tensor_add(rz_sb[:], rz_sb[:], brz_sb[:, 4 * co4 : 4 * co4 + 4])
                nc.scalar.activation(
                    rz_sb[:], rz_sb[:], mybir.ActivationFunctionType.Sigmoid)
                # n = tanh(gi_n + b_in + r*(gh_n + b_hn))
                hnb_sb = sb.tile([P, 2], F32, tag="hnbc")
                nc.vector.tensor_add(hnb_sb[:], gh_sb[:, 4:6], b_hn_sb[:, cs])
                nc.vector.tensor_mul(hnb_sb[:], hnb_sb[:], rz_sb[:, 0:2])
                npre_sb = sb.tile([P, 2], F32, tag="nprec")
                nc.vector.tensor_add(npre_sb[:], gi_ps[:, 4:6], hnb_sb[:])
                nc.vector.tensor_add(npre_sb[:], npre_sb[:], b_in_sb[:, cs])
                n_sb = sb.tile([P, 2], F32, tag="nc")
                nc.scalar.activation(
                    n_sb[:], npre_sb[:], mybir.ActivationFunctionType.Tanh)
                # h_new = n + z*(h - n)
                hmn_sb = sb.tile([P, 2], F32, tag="hmnc")
                nc.vector.tensor_sub(hmn_sb[:], hprev_sb[:, cs], n_sb[:])
                nc.vector.tensor_mul(hmn_sb[:], hmn_sb[:], rz_sb[:, 2:4])
                nc.vector.tensor_add(hnew_sb[:, cs], n_sb[:], hmn_sb[:])
                if OUT_FP8:
                    nc.vector.tensor_scalar_mul(
                        hnew_w_sb[:, cs], hnew_sb[:, cs], H_SCALE)
                else:
                    nc.vector.tensor_copy(hnew_w_sb[:, cs], hnew_sb[:, cs])

                # vocab projection for this pair
                part_ps = psum.tile([P, VC], F32, tag="plog", bufs=2)
                for vc in range(VC):
                    nc.tensor.matmul(
                        part_ps[:, vc : vc + 1],
                        w_sb[co4][:, :, vc * P : (vc + 1) * P],
                        hnew_w_sb[:, cs].rearrange("p (two n) -> p two n", n=1),
                        start=True,
                        stop=True,
                        perf_mode=mybir.MatmulPerfMode.DoubleRow,
                    )
                if co4 == 0:
                    nc.vector.scalar_tensor_tensor(
                        logits_sb[:], part_ps[:], UNSCALE, out_b_sb[:],
                        op0=mybir.AluOpType.mult, op1=mybir.AluOpType.add,
                    )
                else:
                    nc.vector.scalar_tensor_tensor(
                        logits_sb[:], part_ps[:], UNSCALE, logits_sb[:],
                        op0=mybir.AluOpType.mult, op1=mybir.AluOpType.add,
                    )
            nc.sync.dma_start(hnew_out, hnew_sb[:])
            nc.sync.dma_start(logits_out, logits_sb[:])

            # ---- local exp-sum for the host-side log-softmax -------------
            exp_sb = sb.tile([P, VC], F32)
            srow_sb = sb.tile([P, 1], F32)
            nc.scalar.activation(
                exp_sb[:], logits_sb[:], mybir.ActivationFunctionType.Exp,
                accum_out=srow_sb[:],
            )
            ssum_ps = psum.tile([1, 1], F32, tag="pb")
            nc.tensor.matmul(ssum_ps[:], srow_sb[:], ones[:, 0:1], start=True, stop=True)
            ssum_sb = sb.tile([1, 1], F32)
            nc.scalar.copy(ssum_sb[:], ssum_ps[:])
            nc.sync.dma_start(ssum_out, ssum_sb[:])

    nc.compile()
    return nc


_NC_CACHE = None


def _get_nc():
    global _NC_CACHE
    if _NC_CACHE is None:
        _NC_CACHE = build_nc()
    return _NC_CACHE


def _pc(v):
    """[1024] -> [128, 8] with v[c*128+p] at [p, c]."""
    return np.ascontiguousarray(v.reshape(KC, P).T)


def make_in_maps(prev_word, prev_hidden, encoder_outputs, emb, attn_W, attn_b,
                 comb_W, comb_b, w_ih, w_hh, b_ih, b_hh, out_W, out_b):
    f32 = lambda a: np.asarray(a, dtype=np.float32)
    idx = int(np.asarray(prev_word).reshape(-1)[0])
    emb_row = f32(emb)[idx].reshape(H)
    hprev = f32(prev_hidden).reshape(H)
    attn_W = f32(attn_W)
    attn_b = f32(attn_b)
    enc = np.ascontiguousarray(f32(encoder_outputs))
    comb_W = f32(comb_W)
    comb_b = f32(comb_b)
    w_ih, w_hh, b_ih, b_hh = f32(w_ih), f32(w_hh), f32(b_ih), f32(b_hh)
    out_W, out_b = f32(out_W), f32(out_b)

    # replicated tensors (same arrays for every core)
    rep = {
        "emb_pc": _pc(emb_row),
        "hprev_pc": _pc(hprev),
        # [p, c*L+m] = attn_W[m, c*128+p]
        "attn_wt": np.ascontiguousarray(
            attn_W.T.reshape(16, P, L).transpose(1, 0, 2).reshape(P, 16 * L)),
        "attn_b": np.ascontiguousarray(attn_b.reshape(L, 1)),
        "enc": enc,
        # [p, (co*16+ck)*128+m] = comb_W[co*128+m, ck*128+p]
        "comb_wt": np.ascontiguousarray(
            comb_W.reshape(KC, P, 16, P).transpose(3, 0, 2, 1)
            .reshape(P, 16 * KC * P)).astype(WDT_NP),
        "comb_b": _pc(comb_b),
        # pair-major: [p, (((co4*3+g)*2+cc)*8+kc)*128+m] = W[g*1024+(2*co4+cc)*128+m, kc*128+p]
        "wih_t": np.ascontiguousarray(
            w_ih.reshape(3, KC // 2, 2, P, KC, P).transpose(5, 1, 0, 2, 4, 3)
            .reshape(P, 3 * KC * H)).astype(WDT_NP),
        "whh_t": np.ascontiguousarray(
            w_hh.reshape(3, KC // 2, 2, P, KC, P).transpose(5, 1, 0, 2, 4, 3)
            .reshape(P, 3 * KC * H)).astype(WDT_NP),
        # pair-major r/z bias: block co4 = [r_{2co4}, r_{2co4+1}, z_{2co4}, z_{2co4+1}]
        "brz": np.ascontiguousarray(np.concatenate(
            [_pc((b_ih + b_hh)[:H]).reshape(P, KC // 2, 2),
             _pc((b_ih + b_hh)[H : 2 * H]).reshape(P, KC // 2, 2)],
            axis=2).reshape(P, 2 * KC)),
        "b_in": _pc(b_ih[2 * H :]),
        "b_hn": _pc(b_hh[2 * H :]),
    }

    Wp = np.zeros((VP, H), np.float32)
    Wp[:V] = out_W
    bp = np.full(VP, NEG_BIG, np.float32)
    bp[:V] = out_b

    in_maps = []
    for i in range(N_CORES):
        vsl = slice(VS * i, VS * (i + 1))
        m = dict(rep)
        m["out_wt"] = np.ascontiguousarray(
            np.clip(Wp[vsl].T * W_SCALE, -240.0, 240.0)
            .reshape(KC // 2, 2, P, VS).transpose(0, 2, 1, 3)
        ).astype(ODT_NP)
        m["out_b"] = np.ascontiguousarray(bp[vsl].reshape(VC, P).T)
        in_maps.append(m)
    return in_maps


LAST_RESULTS = None


def kernel(**inputs):
    global LAST_RESULTS
    nc = _get_nc()
    in_maps = make_in_maps(**inputs)
    trace = bool(int(os.environ.get("KERNEL_TRACE", "0")))
    res = bass_utils.run_bass_kernel_spmd(
        nc, in_maps, core_ids=list(range(N_CORES)), trace=trace,
    )
    LAST_RESULTS = res
    logits = np.concatenate(
        [np.asarray(r["logits"]).T.reshape(VS) for r in res.results])
    s_total = float(sum(np.asarray(r["ssum"]).reshape(()) for r in res.results))
    probs = (logits[:V] - np.float32(np.log(s_total))).reshape(1, V)
    h_new = np.asarray(res.results[0]["h_new"]).T.reshape(1, 1, H)
    return probs.astype(np.float32), h_new.astype(np.float32)


# revision 27
# speedup vs baseline: 1.1322x; 1.0487x over previous
"""Trainium2 Bass kernel for a single-step seq2seq GRU decoder with attention.

Computation (batch=1):
  embedded = emb[prev_word]                                      [1, H]
  attn_w   = softmax([embedded, h_prev] @ attn_W.T + attn_b)     [1, L]
  focused  = attn_w @ encoder_outputs                            [1, H]
  gru_in   = relu([focused, embedded] @ comb_W.T + comb_b)       [1, H]
  h_new    = GRU(gru_in, h_prev)                                 [1, H]
  probs    = log_softmax(h_new @ out_W.T + out_b)                [1, V]

Distribution over 8 NeuronCores, with NO cross-core collectives:
  - the vocab dim of out_W/out_b (the 206MB input that dominates the
    memory-bound roofline) is sharded 8 ways
  - the small attention/comb/GRU stages are replicated on every core
    (batch=1: cheaper than paying a cross-core sync for their shards)
  - log-softmax: each core emits its local exp-sum; the host unshard step
    combines the 8 scalars and subtracts log(S) while concatenating.
  Collectives are deliberately avoided: a NEFF with collectives pays a
  multi-core rendezvous at entry, which costs the full inter-core dispatch
  skew on every execution.

On-device layout: a hidden vector x[1024] lives as [128, 8] SBUF tiles with
x[c*128+p] at [p, c] (partition-parallel everywhere; no transposes needed).
"""

import os
import numpy as np
import ml_dtypes

import concourse.bass as bass
import concourse.bacc as bacc
import concourse.mybir as mybir
import concourse.tile as tile
import concourse.bass_utils as bass_utils

V, H, L = 50257, 1024, 20
N_CORES = 8
P = 128
KC = H // P            # 8 hidden chunks of 128
VP = 51200             # padded vocab = 8 * 6400
VS = VP // N_CORES     # 6400 vocab rows per core
VC = VS // P           # 50 vocab chunks of 128

F32 = mybir.dt.float32
BF16 = mybir.dt.bfloat16
NEG_BIG = -1.0e30

# dtype of the replicated comb/GRU weights (WDT) and the out_W shard (ODT).
# GRU stays bf16 (fp8 would push h_new error to ~1.5e-2); out_W tolerates
# fp8 e4m3 with a 256x scale (probs error ~3e-3 vs bf16's 3e-4).
WDT = BF16
WDT_NP = ml_dtypes.bfloat16
OUT_FP8 = True
ODT = mybir.dt.float8e4 if OUT_FP8 else BF16
ODT_NP = ml_dtypes.float8_e4m3 if OUT_FP8 else ml_dtypes.bfloat16
W_SCALE = 256.0 if OUT_FP8 else 1.0   # fp8 quantization scale for out_W
H_SCALE = 16.0 if OUT_FP8 else 1.0    # fp8 quantization scale for h_new rhs
UNSCALE = 1.0 / (W_SCALE * H_SCALE)


def build_nc():
    nc = bacc.Bacc(
        "TRN2",
        target_bir_lowering=False,
        debug=False,
        enable_asserts=False,
        num_devices=N_CORES,
    )

    def inp(name, shape, dt=F32):
        return nc.dram_tensor(name, shape, dt, kind="ExternalInput").ap()

    # replicated inputs
    emb_pc = inp("emb_pc", [P, KC])            # embedded word, (c p) -> p c
    hprev_pc = inp("hprev_pc", [P, KC])        # h_prev, (c p) -> p c
    attn_wt = inp("attn_wt", [P, 16 * L])      # attn_W.T chunked [p, c*L+m]
    attn_b = inp("attn_b", [L, 1])
    enc = inp("enc", [L, H])                   # encoder_outputs
    comb_wt = inp("comb_wt", [P, 16 * KC * P], WDT)  # [p,(co*16+ck)*128+m]
    comb_b = inp("comb_b", [P, KC])
    wih_t = inp("wih_t", [P, 3 * KC * H], WDT)  # [p,((g*8+co)*8+kc)*128+m]
    whh_t = inp("whh_t", [P, 3 * KC * H], WDT)
    brz = inp("brz", [P, 2 * KC])              # (b_ih+b_hh) r,z in (c p)
    b_in = inp("b_in", [P, KC])                # b_ih n slice
    b_hn = inp("b_hn", [P, KC])                # b_hh n slice
    # sharded inputs. out_wt is packed for fp8 DoubleRow: pair-chunk kc4
    # holds hidden rows [256*kc4, 256*kc4+256) as [p, i, v] = row 128i+p.
    out_wt = inp("out_wt", [KC // 2, P, 2, VS], ODT)
    out_b = inp("out_b", [P, VC])              # bias shard [p, vc]

    logits_out = nc.dram_tensor("logits", [P, VC], F32, kind="ExternalOutput").ap()
    ssum_out = nc.dram_tensor("ssum", [1, 1], F32, kind="ExternalOutput").ap()
    hnew_out = nc.dram_tensor("h_new", [P, KC], F32, kind="ExternalOutput").ap()

    with tile.TileContext(nc) as tc:
        with (
            tc.tile_pool(name="consts", bufs=1) as consts,
            tc.tile_pool(name="sb", bufs=1) as sb,
            tc.tile_pool(name="wpool", bufs=1) as wpool,
            tc.tile_pool(name="psum", bufs=1, space="PSUM") as psum,
        ):
            ones = consts.tile([P, P], F32)
            nc.vector.memset(ones[:], 1.0)
            # warm the ACT function tables off the critical path (each first
            # use of Sigmoid/Tanh/Exp otherwise pays a ~1.3us table load)
            warm = consts.tile([1, 3], F32)
            for wi, fn in enumerate((mybir.ActivationFunctionType.Sigmoid,
                                     mybir.ActivationFunctionType.Tanh,
                                     mybir.ActivationFunctionType.Exp)):
                nc.scalar.activation(warm[:, wi : wi + 1], ones[0:1, 0:1], fn)

            # ---- small/critical weights first (DMA order matters) --------
            emb_sb = sb.tile([P, KC], F32)
            nc.sync.dma_start(emb_sb[:], emb_pc)
            hprev_sb = sb.tile([P, KC], F32)
            nc.sync.dma_start(hprev_sb[:], hprev_pc)
            attn_wt_sb = sb.tile([P, 16 * L], F32)
            nc.sync.dma_start(attn_wt_sb[:], attn_wt)
            attn_b_sb = sb.tile([L, 1], F32)
            nc.sync.dma_start(attn_b_sb[:], attn_b)
            enc_sb = sb.tile([L, H], F32)
            nc.sync.dma_start(enc_sb[:], enc)
            comb_b_sb = sb.tile([P, KC], F32)
            nc.sync.dma_start(comb_b_sb[:], comb_b)
            brz_sb = sb.tile([P, 2 * KC], F32)
            nc.sync.dma_start(brz_sb[:], brz)
            b_in_sb = sb.tile([P, KC], F32)
            nc.sync.dma_start(b_in_sb[:], b_in)
            b_hn_sb = sb.tile([P, KC], F32)
            nc.sync.dma_start(b_hn_sb[:], b_hn)
            out_b_sb = sb.tile([P, VC], F32)
            nc.sync.dma_start(out_b_sb[:], out_b)

            # GRU weights in pair-major tiles (all 3 gates for one h_new
            # column pair per tile) and out_W pair-chunks, with DMAs
            # interleaved in pipeline consumption order: the pair-k GRU
            # matmuls and vocab matmuls become runnable as early as possible.
            # column-pair-and-cc granular tiles: each is one DMA that a
            # small block of matmuls consumes, so PE never waits on a big
            # tile completion.
            CSZ = 3 * KC * P  # blocks for one column: 3 gates x 8 k-chunks
            comb_wt_sb = sb.tile([P, 16 * KC * P], WDT)
            whh_c, wih_c, w_sb = [], [], []
            for co4 in range(KC // 2):
                wh2, wi2 = [], []
                for cc in range(2):
                    t = sb.tile([P, CSZ], WDT, tag=f"whh{co4}_{cc}")
                    nc.sync.dma_start(
                        t[:], whh_t[:, (2 * co4 + cc) * CSZ : (2 * co4 + cc + 1) * CSZ])
                    wh2.append(t)
                whh_c.append(wh2)
                if co4 == 0:
                    ch = 16 * KC * P // 2
                    nc.sync.dma_start(comb_wt_sb[:, :ch], comb_wt[:, :ch])
                    nc.sync.dma_start(comb_wt_sb[:, ch:], comb_wt[:, ch:])
                for cc in range(2):
                    t = sb.tile([P, CSZ], WDT, tag=f"wih{co4}_{cc}")
                    nc.sync.dma_start(
                        t[:], wih_t[:, (2 * co4 + cc) * CSZ : (2 * co4 + cc + 1) * CSZ])
                    wi2.append(t)
                wih_c.append(wi2)
                wt = wpool.tile([P, 2, VS], ODT, tag=f"w{co4}")
                vh = VS // 2
                nc.sync.dma_start(wt[:, 0, :vh], out_wt[co4][:, 0, :vh])
                nc.sync.dma_start(wt[:, 1, :vh], out_wt[co4][:, 1, :vh])
                nc.sync.dma_start(wt[:, 0, vh:], out_wt[co4][:, 0, vh:])
                nc.sync.dma_start(wt[:, 1, vh:], out_wt[co4][:, 1, vh:])
                w_sb.append(wt)

            # ---- stage A: attention (replicated, fp32) -------------------
            attnlog_ps = psum.tile([L, 1], F32, tag="pa")
            for c in range(16):
                rhs = emb_sb[:, c : c + 1] if c < KC else hprev_sb[:, c - KC : c - KC + 1]
                nc.tensor.matmul(
                    attnlog_ps[:],
                    attn_wt_sb[:, c * L : (c + 1) * L],
                    rhs,
                    start=(c == 0),
                    stop=(c == 15),
                )
            # exp(logit + b); logits are tiny so no max-subtraction needed
            expw_sb = sb.tile([L, 1], F32)
            nc.scalar.activation(
                expw_sb[:], attnlog_ps[:], mybir.ActivationFunctionType.Exp,
                bias=attn_b_sb[:],
            )
            asum_ps = psum.tile([1, 1], F32, tag="pb")
            nc.tensor.matmul(asum_ps[:], expw_sb[:], ones[:L, 0:1], start=True, stop=True)
            arecip_sb = sb.tile([1, 1], F32)
            nc.vector.reciprocal(arecip_sb[:], asum_ps[:])
            # focused (unnormalized) [128, KC]
            foc_ps = psum.tile([P, KC], F32, tag="pm")
            for c in range(KC):
                nc.tensor.matmul(
                    foc_ps[:, c : c + 1],
                    enc_sb[:, c * P : (c + 1) * P],
                    expw_sb[:],
                    start=True,
                    stop=True,
                )
            # broadcast 1/denom across partitions via PE ones column
            arb_ps = psum.tile([P, 1], F32, tag="pb")
            nc.tensor.matmul(arb_ps[:], ones[0:1, :], arecip_sb[:], start=True, stop=True)
            arb_sb = sb.tile([P, 1], F32)
            nc.scalar.copy(arb_sb[:], arb_ps[:])
            fsc_sb = sb.tile([P, KC], WDT)
            nc.vector.tensor_scalar_mul(fsc_sb[:], foc_ps[:], arb_sb[:])
            emb_w_sb = sb.tile([P, KC], WDT)
            nc.vector.tensor_copy(emb_w_sb[:], emb_sb[:])

            # ---- stage B: comb (replicated) -> gru_in [128, KC] ----------
            gcol_ps = psum.tile([P, KC], F32, tag="pm")
            for co in range(KC):
                for ck in range(16):
                    rhs = (fsc_sb[:, ck : ck + 1] if ck < KC
                           else emb_w_sb[:, ck - KC : ck - KC + 1])
                    nc.tensor.matmul(
                        gcol_ps[:, co : co + 1],
                        comb_wt_sb[:, (co * 16 + ck) * P : (co * 16 + ck + 1) * P],
                        rhs,
                        start=(ck == 0),
                        stop=(ck == 15),
                    )
            gin_f_sb = sb.tile([P, KC], F32)
            nc.vector.tensor_add(gin_f_sb[:], gcol_ps[:], comb_b_sb[:])
            nc.vector.tensor_relu(gin_f_sb[:], gin_f_sb[:])
            gin_sb = sb.tile([P, KC], WDT)
            nc.vector.tensor_copy(gin_sb[:], gin_f_sb[:])
            hprev_w_sb = sb.tile([P, KC], WDT)
            nc.vector.tensor_copy(hprev_w_sb[:], hprev_sb[:])

            # ---- stages C+D pipelined per h_new column pair --------------
            # For each pair of h_new columns (2*co4, 2*co4+1): run the GRU
            # matmuls and gate math for just those columns, then immediately
            # the 50 DoubleRow vocab matmuls that consume them. The vocab
            # projection overlaps the remaining GRU work instead of waiting
            # for the full h_new.
            hnew_sb = sb.tile([P, KC], F32)
            hnew_w_sb = sb.tile([P, KC], ODT)
            logits_sb = sb.tile([P, VC], F32)
            for co4 in range(KC // 2):
                cs = slice(2 * co4, 2 * co4 + 2)
                # gi/gh for the pair: psum [P, 6], col j = g*2 + cc
                gh_ps = psum.tile([P, 6], F32, tag="pgh")
                gi_ps = psum.tile([P, 6], F32, tag="pgi")
                for cc in range(2):
                    for g in range(3):
                        for kc in range(KC):
                            off = (g * KC + kc) * P
                            nc.tensor.matmul(
                                gh_ps[:, g * 2 + cc : g * 2 + cc + 1],
                                whh_c[co4][cc][:, off : off + P],
                                hprev_w_sb[:, kc : kc + 1],
                                start=(kc == 0),
                                stop=(kc == KC - 1),
                            )
                for cc in range(2):
                    for g in range(3):
                        for kc in range(KC):
                            off = (g * KC + kc) * P
                            nc.tensor.matmul(
                                gi_ps[:, g * 2 + cc : g * 2 + cc + 1],
                                wih_c[co4][cc][:, off : off + P],
                                gin_sb[:, kc : kc + 1],
                                start=(kc == 0),
                                stop=(kc == KC - 1),
                            )
                gh_sb = sb.tile([P, 6], F32, tag="ghc")
                nc.scalar.copy(gh_sb[:], gh_ps[:])
                # r,z = sigmoid(gi + gh + brz) (cols 0:4 = r0 r1 z0 z1)
                rz_sb = sb.tile([P, 4], F32, tag="rzc")
                nc.vector.tensor_add(rz_sb[:], gi_ps[:, 0:4], gh_sb[:, 0:4])
                nc.vector.tensor_add(rz_sb[:], rz_sb[:], brz_sb[:, 4 * co4 : 4 * co4 + 4])
                nc.scalar.activation(
                    rz_sb[:], rz_sb[:], mybir.ActivationFunctionType.Sigmoid)
                # n = tanh(gi_n + b_in + r*(gh_n + b_hn))
                hnb_sb = sb.tile([P, 2], F32, tag="hnbc")
                nc.vector.tensor_add(hnb_sb[:], gh_sb[:, 4:6], b_hn_sb[:, cs])
                nc.vector.tensor_mul(hnb_sb[:], hnb_sb[:], rz_sb[:, 0:2])
                npre_sb = sb.tile([P, 2], F32, tag="nprec")
                nc.vector.tensor_add(npre_sb[:], gi_ps[:, 4:6], hnb_sb[:])
                nc.vector.tensor_add(npre_sb[:], npre_sb[:], b_in_sb[:, cs])
                n_sb = sb.tile([P, 2], F32, tag="nc")
                nc.scalar.activation(
                    n_sb[:], npre_sb[:], mybir.ActivationFunctionType.Tanh)
                # h_new = n + z*(h - n)
                hmn_sb = sb.tile([P, 2], F32, tag="hmnc")
                nc.vector.tensor_sub(hmn_sb[:], hprev_sb[:, cs], n_sb[:])
                nc.vector.tensor_mul(hmn_sb[:], hmn_sb[:], rz_sb[:, 2:4])
                nc.vector.tensor_add(hnew_sb[:, cs], n_sb[:], hmn_sb[:])
                if OUT_FP8:
                    nc.vector.tensor_scalar_mul(
                        hnew_w_sb[:, cs], hnew_sb[:, cs], H_SCALE)
                else:
                    nc.vector.tensor_copy(hnew_w_sb[:, cs], hnew_sb[:, cs])

                # vocab projection for this pair
                part_ps = psum.tile([P, VC], F32, tag="plog", bufs=2)
                for vc in range(VC):
                    nc.tensor.matmul(
                        part_ps[:, vc : vc + 1],
                        w_sb[co4][:, :, vc * P : (vc + 1) * P],
                        hnew_w_sb[:, cs].rearrange("p (two n) -> p two n", n=1),
                        start=True,
                        stop=True,
                        perf_mode=mybir.MatmulPerfMode.DoubleRow,
                    )
                if co4 == 0:
                    nc.vector.scalar_tensor_tensor(
                        logits_sb[:], part_ps[:], UNSCALE, out_b_sb[:],
                        op0=mybir.AluOpType.mult, op1=mybir.AluOpType.add,
                    )
                else:
                    nc.vector.scalar_tensor_tensor(
                        logits_sb[:], part_ps[:], UNSCALE, logits_sb[:],
                        op0=mybir.AluOpType.mult, op1=mybir.AluOpType.add,
                    )
            nc.sync.dma_start(hnew_out, hnew_sb[:])
            nc.sync.dma_start(logits_out, logits_sb[:])

            # ---- local exp-sum for the host-side log-softmax -------------
            exp_sb = sb.tile([P, VC], F32)
            srow_sb = sb.tile([P, 1], F32)
            nc.scalar.activation(
                exp_sb[:], logits_sb[:], mybir.ActivationFunctionType.Exp,
                accum_out=srow_sb[:],
            )
            ssum_ps = psum.tile([1, 1], F32, tag="pb")
            nc.tensor.matmul(ssum_ps[:], srow_sb[:], ones[:, 0:1], start=True, stop=True)
            ssum_sb = sb.tile([1, 1], F32)
            nc.scalar.copy(ssum_sb[:], ssum_ps[:])
            nc.sync.dma_start(ssum_out, ssum_sb[:])

    nc.compile()
    return nc


_NC_CACHE = None


def _get_nc():
    global _NC_CACHE
    if _NC_CACHE is None:
        _NC_CACHE = build_nc()
    return _NC_CACHE


def _pc(v):
    """[1024] -> [128, 8] with v[c*128+p] at [p, c]."""
    return np.ascontiguousarray(v.reshape(KC, P).T)


def make_in_maps(prev_word, prev_hidden, encoder_outputs, emb, attn_W, attn_b,
                 comb_W, comb_b, w_ih, w_hh, b_ih, b_hh, out_W, out_b):
    f32 = lambda a: np.asarray(a, dtype=np.float32)
    idx = int(np.asarray(prev_word).reshape(-1)[0])
    emb_row = f32(emb)[idx].reshape(H)
    hprev = f32(prev_hidden).reshape(H)
    attn_W = f32(attn_W)
    attn_b = f32(attn_b)
    enc = np.ascontiguousarray(f32(encoder_outputs))
    comb_W = f32(comb_W)
    comb_b = f32(comb_b)
    w_ih, w_hh, b_ih, b_hh = f32(w_ih), f32(w_hh), f32(b_ih), f32(b_hh)
    out_W, out_b = f32(out_W), f32(out_b)

    # replicated tensors (same arrays for every core)
    rep = {
        "emb_pc": _pc(emb_row),
        "hprev_pc": _pc(hprev),
        # [p, c*L+m] = attn_W[m, c*128+p]
        "attn_wt": np.ascontiguousarray(
            attn_W.T.reshape(16, P, L).transpose(1, 0, 2).reshape(P, 16 * L)),
        "attn_b": np.ascontiguousarray(attn_b.reshape(L, 1)),
        "enc": enc,
        # [p, (co*16+ck)*128+m] = comb_W[co*128+m, ck*128+p]
        "comb_wt": np.ascontiguousarray(
            comb_W.reshape(KC, P, 16, P).transpose(3, 0, 2, 1)
            .reshape(P, 16 * KC * P)).astype(WDT_NP),
        "comb_b": _pc(comb_b),
        # column-major: [p, (((co4*2+cc)*3+g)*8+kc)*128+m] = W[g*1024+(2*co4+cc)*128+m, kc*128+p]
        "wih_t": np.ascontiguousarray(
            w_ih.reshape(3, KC // 2, 2, P, KC, P).transpose(5, 1, 2, 0, 4, 3)
            .reshape(P, 3 * KC * H)).astype(WDT_NP),
        "whh_t": np.ascontiguousarray(
            w_hh.reshape(3, KC // 2, 2, P, KC, P).transpose(5, 1, 2, 0, 4, 3)
            .reshape(P, 3 * KC * H)).astype(WDT_NP),
        # pair-major r/z bias: block co4 = [r_{2co4}, r_{2co4+1}, z_{2co4}, z_{2co4+1}]
        "brz": np.ascontiguousarray(np.concatenate(
            [_pc((b_ih + b_hh)[:H]).reshape(P, KC // 2, 2),
             _pc((b_ih + b_hh)[H : 2 * H]).reshape(P, KC // 2, 2)],
            axis=2).reshape(P, 2 * KC)),
        "b_in": _pc(b_ih[2 * H :]),
        "b_hn": _pc(b_hh[2 * H :]),
    }

    Wp = np.zeros((VP, H), np.float32)
    Wp[:V] = out_W
    bp = np.full(VP, NEG_BIG, np.float32)
    bp[:V] = out_b

    in_maps = []
    for i in range(N_CORES):
        vsl = slice(VS * i, VS * (i + 1))
        m = dict(rep)
        m["out_wt"] = np.ascontiguousarray(
            np.clip(Wp[vsl].T * W_SCALE, -240.0, 240.0)
            .reshape(KC // 2, 2, P, VS).transpose(0, 2, 1, 3)
        ).astype(ODT_NP)
        m["out_b"] = np.ascontiguousarray(bp[vsl].reshape(VC, P).T)
        in_maps.append(m)
    return in_maps


LAST_RESULTS = None


def kernel(**inputs):
    global LAST_RESULTS
    nc = _get_nc()
    in_maps = make_in_maps(**inputs)
    trace = bool(int(os.environ.get("KERNEL_TRACE", "0")))
    res = bass_utils.run_bass_kernel_spmd(
        nc, in_maps, core_ids=list(range(N_CORES)), trace=trace,
    )
    LAST_RESULTS = res
    logits = np.concatenate(
        [np.asarray(r["logits"]).T.reshape(VS) for r in res.results])
    s_total = float(sum(np.asarray(r["ssum"]).reshape(()) for r in res.results))
    probs = (logits[:V] - np.float32(np.log(s_total))).reshape(1, V)
    h_new = np.asarray(res.results[0]["h_new"]).T.reshape(1, 1, H)
    return probs.astype(np.float32), h_new.astype(np.float32)
